# revision 4
# baseline (speedup 1.0000x reference)
"""MASKGCN Trainium2 kernel: 3-layer masked GCN over B=512 graphs of N=200 nodes.

Strategy
--------
Data-parallel over the batch: 64 graphs per NeuronCore, 8 cores, no collectives.

Math fold (exact up to fp reassociation):
    mask = (E + E^T)/2 + I                 (host)
    A    = sigmoid(adj) * mask             (device; adj is 0/1 so
                                            sigmoid(adj) = c*(adj + s), s = 0.5/c,
                                            c = sigmoid(1) - 0.5)
    S0 = F @ W0 ; H1 = A @ S0 ; S1 = H1 @ W1 ; H2 = A @ S1
    out_g = (1/200) * colsum(A)^T @ (H2 @ (W2 @ pw)) + pb
The whole last GCN layer + mean readout + linear head collapse into a
[200]-vector / [256,2]-matrix contraction (colsum(A) is a free-axis reduction
of A^T, fused into the same DVE op that builds A^T).

Layouts: host passes adj^T and F^T per graph. A^T = sigmoid(adj^T) * mask
(mask symmetric). The chain alternates normal/transposed layouts so that NO
on-chip transposes are needed:
    S0  (normal  [node, h])  = matmul(lhsT=F^T slices,  rhs=W0)
    H1t (transp. [h, node])  = matmul(lhsT=S0 slices,   rhs=A^T)
    S1  (normal)             = matmul(lhsT=H1t slices,  rhs=W1)
    H2t (transp.)            = matmul(lhsT=S1 slices,   rhs=A^T)
    S2p (normal [node, 2])   = matmul(lhsT=H2t slices,  rhs=Wp)   Wp=(W2@pw)/200
    og  ([1, 2])             = matmul(lhsT=w,           rhs=S2p)  w=colsum(A)
"""

import os
import sys
import numpy as np

# concourse is normally pre-imported by the axon sitecustomize; these are
# fallbacks for environments where it is not on the default path.
if "concourse" not in sys.modules:
    try:
        import concourse  # noqa: F401
    except ImportError:
        for _p in ("/opt/trn_rl_repo", "/root/.axon_site/_ro/trn_rl_repo"):
            if os.path.isdir(_p) and _p not in sys.path:
                sys.path.append(_p)

B, N, IN_C, HID, OUT_C, N_VARS = 512, 200, 200, 256, 256, 2
N_CORES = 8
BPC = B // N_CORES  # graphs per core
P0 = 128
P1 = N - P0  # 72

# sigmoid(adj) = C_SIG * (adj + S_SIG) for adj in {0, 1}
C_SIG = float(1.0 / (1.0 + np.exp(-1.0)) - 0.5)  # 0.23105857863000487
S_SIG = float(0.5 / C_SIG)                       # 2.1639534137386535

_BUILD_CACHE = {}


def _build_nc(bpc, reps=1):
    """Build the per-core Bass program (SPMD: identical on all cores).

    reps>1 wraps the whole batch loop in a hardware For_i so the body runs
    `reps` times — benchmarking only (differencing cancels dispatch floor)."""
    import concourse.bacc as bacc
    import concourse.mybir as mybir
    import concourse.tile as tile
    from contextlib import ExitStack, nullcontext

    f32 = mybir.dt.float32
    ADD = mybir.AluOpType.add
    MULT = mybir.AluOpType.mult

    nc = bacc.Bacc(None, target_bir_lowering=False)
    adjt = nc.declare_dram_parameter("adjt", [bpc, N, N], f32, isOutput=False)
    ft = nc.declare_dram_parameter("ft", [bpc, N, N], f32, isOutput=False)
    maska = nc.declare_dram_parameter("maska", [N, N], f32, isOutput=False)
    w0 = nc.declare_dram_parameter("w0", [IN_C, HID], f32, isOutput=False)
    w1 = nc.declare_dram_parameter("w1", [HID, HID], f32, isOutput=False)
    wp = nc.declare_dram_parameter("wp", [OUT_C, N_VARS], f32, isOutput=False)
    out = nc.declare_dram_parameter("out", [1, bpc * N_VARS], f32, isOutput=True)

    with tile.TileContext(nc) as tc, ExitStack() as ctx:
        consts = ctx.enter_context(tc.tile_pool(name="consts", bufs=1))
        inp = ctx.enter_context(tc.tile_pool(name="inp", bufs=3))
        atp = ctx.enter_context(tc.tile_pool(name="atp", bufs=2))
        sp = ctx.enter_context(tc.tile_pool(name="sp", bufs=2))
        htp = ctx.enter_context(tc.tile_pool(name="htp", bufs=2))
        smallp = ctx.enter_context(tc.tile_pool(name="smallp", bufs=2))
        pstage = ctx.enter_context(
            tc.tile_pool(name="pstage", bufs=1, space="PSUM")
        )
        psmall = ctx.enter_context(
            tc.tile_pool(name="psmall", bufs=2, space="PSUM")
        )

        # ---- constants (loaded once) ----
        w0a = consts.tile([P0, HID], f32, tag="w0a")
        w0b = consts.tile([P1, HID], f32, tag="w0b")
        w1a = consts.tile([P0, HID], f32, tag="w1a")
        w1b = consts.tile([P0, HID], f32, tag="w1b")
        wpa = consts.tile([P0, N_VARS], f32, tag="wpa")
        wpb = consts.tile([P0, N_VARS], f32, tag="wpb")
        mka = consts.tile([P0, N], f32, tag="mka")
        mkb = consts.tile([P1, N], f32, tag="mkb")
        out_acc = consts.tile([1, bpc * N_VARS], f32, tag="out_acc")
        nc.sync.dma_start(w0a[:], w0[0:P0, :])
        nc.sync.dma_start(w0b[:], w0[P0:N, :])
        nc.sync.dma_start(w1a[:], w1[0:P0, :])
        nc.sync.dma_start(w1b[:], w1[P0:HID, :])
        nc.sync.dma_start(wpa[:], wp[0:P0, :])
        nc.sync.dma_start(wpb[:], wp[P0:OUT_C, :])
        nc.sync.dma_start(mka[:], maska[0:P0, :])
        nc.sync.dma_start(mkb[:], maska[P0:N, :])
        w0_t = (w0a, w0b)
        w1_t = (w1a, w1b)
        wp_t = (wpa, wpb)

        mslc = ((0, P0), (P0, P1))  # node-dim (offset, count) tiles

        def emit_batch():
            for g in range(bpc):
                # ---- load adj^T and F^T tiles ----
                a0 = inp.tile([P0, N], f32, tag="a0")
                a1 = inp.tile([P1, N], f32, tag="a1")
                f0 = inp.tile([P0, N], f32, tag="f0")
                f1 = inp.tile([P1, N], f32, tag="f1")
                nc.sync.dma_start(a0[:], adjt[g, 0:P0, :])
                nc.sync.dma_start(a1[:], adjt[g, P0:N, :])
                nc.sync.dma_start(f0[:], ft[g, 0:P0, :])
                nc.sync.dma_start(f1[:], ft[g, P0:N, :])

                # ---- A^T = (adj^T + s) * (c*mask); w = rowsum(A^T) fused ----
                at0 = atp.tile([P0, N], f32, tag="at0")
                at1 = atp.tile([P1, N], f32, tag="at1")
                wv = atp.tile([P0, 2], f32, tag="wv")
                nc.vector.scalar_tensor_tensor(
                    at0[:], a0[:], S_SIG, mka[:], op0=ADD, op1=MULT,
                    accum_out=wv[:, 0:1],
                )
                nc.vector.scalar_tensor_tensor(
                    at1[:], a1[:], S_SIG, mkb[:], op0=ADD, op1=MULT,
                    accum_out=wv[0:P1, 1:2],
                )
                at_t = (at0, at1)

                # ---- S0 = F @ W0  -> psum [node, 2*HID] ----
                ps0 = pstage.tile([P0, 2 * HID], f32, tag="ps0")
                f_t = (f0, f1)
                for j, (mo, mc) in enumerate(mslc):
                    for k in range(2):
                        nc.tensor.matmul(
                            ps0[0:mc, j * HID:(j + 1) * HID],
                            f_t[k][:, mo:mo + mc],
                            w0_t[k][:],
                            start=(k == 0), stop=(k == 1),
                        )
                s0a = sp.tile([P0, HID], f32, tag="s0a")
                s0b = sp.tile([P1, HID], f32, tag="s0b")
                nc.vector.tensor_copy(s0a[:], ps0[:, 0:HID])
                nc.vector.tensor_copy(s0b[:], ps0[0:P1, HID:2 * HID])
                s0_t = (s0a, s0b)

                # ---- H1^T = matmul(lhsT=S0 slices, rhs=A^T) -> psum [h, 2*N] ----
                ph1 = pstage.tile([P0, 2 * N], f32, tag="ph1")
                for j in range(2):  # h slice
                    for k in range(2):  # node contraction tile
                        nc.tensor.matmul(
                            ph1[:, j * N:(j + 1) * N],
                            s0_t[k][:, j * P0:(j + 1) * P0],
                            at_t[k][:],
                            start=(k == 0), stop=(k == 1),
                        )
                h1a = htp.tile([P0, N], f32, tag="h1a")
                h1b = htp.tile([P0, N], f32, tag="h1b")
                nc.scalar.copy(h1a[:], ph1[:, 0:N])
                nc.scalar.copy(h1b[:], ph1[:, N:2 * N])
                h1_t = (h1a, h1b)

                # ---- S1 = H1 @ W1 -> psum [node, 2*HID] ----
                ps1 = pstage.tile([P0, 2 * HID], f32, tag="ps1")
                for j, (mo, mc) in enumerate(mslc):
                    for k in range(2):
                        nc.tensor.matmul(
                            ps1[0:mc, j * HID:(j + 1) * HID],
                            h1_t[k][:, mo:mo + mc],
                            w1_t[k][:],
                            start=(k == 0), stop=(k == 1),
                        )
                s1a = sp.tile([P0, HID], f32, tag="s1a")
                s1b = sp.tile([P1, HID], f32, tag="s1b")
                nc.vector.tensor_copy(s1a[:], ps1[:, 0:HID])
                nc.vector.tensor_copy(s1b[:], ps1[0:P1, HID:2 * HID])
                s1_t = (s1a, s1b)

                # ---- H2^T -> psum [h', 2*N] ----
                ph2 = pstage.tile([P0, 2 * N], f32, tag="ph2")
                for j in range(2):
                    for k in range(2):
                        nc.tensor.matmul(
                            ph2[:, j * N:(j + 1) * N],
                            s1_t[k][:, j * P0:(j + 1) * P0],
                            at_t[k][:],
                            start=(k == 0), stop=(k == 1),
                        )
                h2a = htp.tile([P0, N], f32, tag="h2a")
                h2b = htp.tile([P0, N], f32, tag="h2b")
                nc.scalar.copy(h2a[:], ph2[:, 0:N])
                nc.scalar.copy(h2b[:], ph2[:, N:2 * N])
                h2_t = (h2a, h2b)

                # ---- S2p = H2 @ Wp -> psum [node, 2*N_VARS] ----
                ps2 = psmall.tile([P0, 2 * N_VARS], f32, tag="ps2")
                for j, (mo, mc) in enumerate(mslc):
                    for k in range(2):
                        nc.tensor.matmul(
                            ps2[0:mc, j * N_VARS:(j + 1) * N_VARS],
                            h2_t[k][:, mo:mo + mc],
                            wp_t[k][:],
                            start=(k == 0), stop=(k == 1),
                        )
                s2p = smallp.tile([P0, 2 * N_VARS], f32, tag="s2p")
                nc.vector.tensor_copy(s2p[:, 0:N_VARS], ps2[:, 0:N_VARS])
                nc.vector.tensor_copy(
                    s2p[0:P1, N_VARS:2 * N_VARS], ps2[0:P1, N_VARS:2 * N_VARS]
                )

                # ---- og = w^T @ S2p -> psum [1, 2] ----
                og = psmall.tile([1, N_VARS], f32, tag="og")
                nc.tensor.matmul(
                    og[:], wv[:, 0:1], s2p[:, 0:N_VARS], start=True, stop=False
                )
                nc.tensor.matmul(
                    og[:], wv[0:P1, 1:2], s2p[0:P1, N_VARS:2 * N_VARS],
                    start=False, stop=True,
                )
                nc.vector.tensor_copy(
                    out_acc[:, g * N_VARS:(g + 1) * N_VARS], og[:]
                )

        if reps > 1:
            with tc.For_i(0, reps, 1):
                emit_batch()
        else:
            emit_batch()

        nc.sync.dma_start(out[:], out_acc[:])

    nc.compile()
    return nc


def _host_prep(adj, features, raw_edge_weight, W0, W1, W2, pw, pb):
    """Host-side weight prep + per-core input shards."""
    mask = ((raw_edge_weight + raw_edge_weight.T) * 0.5 + np.eye(N, dtype=np.float64)).astype(np.float64)
    maska = (C_SIG * mask).astype(np.float32)
    wp = (W2.astype(np.float64) @ pw.astype(np.float64) / float(N)).astype(np.float32)
    in_maps = []
    for c in range(N_CORES):
        sl = slice(c * BPC, (c + 1) * BPC)
        in_maps.append({
            "adjt": np.ascontiguousarray(adj[sl].transpose(0, 2, 1)),
            "ft": np.ascontiguousarray(features[sl].transpose(0, 2, 1)),
            "maska": maska,
            "w0": np.ascontiguousarray(W0.astype(np.float32)),
            "w1": np.ascontiguousarray(W1.astype(np.float32)),
            "wp": wp,
        })
    return in_maps


def _ensure_ntff_hook():
    """Wire the axon NTFF profile hook into antenv.axon_hooks if missing.

    The agent image's antenv package lacks axon_hooks, so bass_utils's
    trace path dies on import. trn_agent_boot has the ctypes hook
    implementation; expose it under the module name bass_utils expects.
    """
    import types

    try:
        from antenv.axon_hooks import get_axon_ntff_profile_hook  # noqa: F401
        return
    except ImportError:
        pass
    try:
        from trn_agent_boot.trn_boot import _ntff_profile_via_ctypes
        hook = _ntff_profile_via_ctypes("/opt/axon/libaxon_pjrt.so")
    except Exception:
        hook = None
    mod = types.ModuleType("antenv.axon_hooks")
    state = {"hook": hook}
    mod.get_axon_ntff_profile_hook = lambda: state["hook"]
    mod.set_axon_ntff_profile_hook = lambda h: state.__setitem__("hook", h)
    sys.modules["antenv.axon_hooks"] = mod
    import antenv

    antenv.axon_hooks = mod


def kernel(adj, features, raw_edge_weight, W0, W1, W2, pw, pb, _trace=False):
    from concourse.bass_utils import run_bass_kernel_spmd

    if _trace:
        _ensure_ntff_hook()

    adj = np.asarray(adj, dtype=np.float32)
    features = np.asarray(features, dtype=np.float32)
    raw_edge_weight = np.asarray(raw_edge_weight, dtype=np.float32)
    W0 = np.asarray(W0, dtype=np.float32)
    W1 = np.asarray(W1, dtype=np.float32)
    W2 = np.asarray(W2, dtype=np.float32)
    pw = np.asarray(pw, dtype=np.float32)
    pb = np.asarray(pb, dtype=np.float32)

    if "nc" not in _BUILD_CACHE:
        _BUILD_CACHE["nc"] = _build_nc(BPC)
    nc = _BUILD_CACHE["nc"]

    in_maps = _host_prep(adj, features, raw_edge_weight, W0, W1, W2, pw, pb)
    res = run_bass_kernel_spmd(
        nc, in_maps, core_ids=list(range(N_CORES)), trace=bool(_trace)
    )
    out = np.concatenate(
        [res.results[c]["out"].reshape(BPC, N_VARS) for c in range(N_CORES)], axis=0
    )
    out = out + pb[None, :].astype(np.float32)
    if _trace:
        return out, res
    return out



# revision 6
# speedup vs baseline: 1.6227x; 1.6227x over previous
"""MASKGCN Trainium2 kernel: 3-layer masked GCN over B=512 graphs of N=200 nodes.

Strategy
--------
Data-parallel over the batch: 64 graphs per NeuronCore, 8 cores, no collectives.

Math fold (exact up to fp reassociation):
    mask = (E + E^T)/2 + I                 (host)
    A    = sigmoid(adj) * mask             (device; adj is 0/1 so
                                            sigmoid(adj) = c*(adj + s), s = 0.5/c,
                                            c = sigmoid(1) - 0.5)
    S0 = F @ W0 ; H1 = A @ S0 ; S1 = H1 @ W1 ; H2 = A @ S1
    out_g = (1/200) * colsum(A)^T @ (H2 @ (W2 @ pw)) + pb
The whole last GCN layer + mean readout + linear head collapse into a
[200]-vector / [256,2]-matrix contraction (colsum(A) is a free-axis reduction
of A^T, fused into the same op that builds A^T).

Precision: all device tensors are fp16 (PE runs fp16 at 1 col/cycle vs
fp32's 4; PSUM accumulation stays fp32). Simulated end-to-end rel-norm
error vs the fp32 reference: ~9e-4.

Layouts: host passes adj^T and F^T per graph. A^T = sigmoid(adj^T) * mask
(mask symmetric). The chain alternates normal/transposed layouts so that NO
on-chip transposes are needed:
    S0  (normal  [node, h])  = matmul(lhsT=F^T slices,  rhs=W0)
    H1t (transp. [h, node])  = matmul(lhsT=S0 slices,   rhs=A^T)
    S1  (normal)             = matmul(lhsT=H1t slices,  rhs=W1)
    H2t (transp.)            = matmul(lhsT=S1 slices,   rhs=A^T)
    S2p (normal [node, 2])   = matmul(lhsT=H2t slices,  rhs=Wp)   Wp=(W2@pw)/200
    og  ([1, 2])             = matmul(lhsT=w,           rhs=S2p)  w=colsum(A)

Engine split: GpSimd builds A^T (SBUF->SBUF, fused colsum accum), Vector
evacuates S0/S1 (PSUM->SBUF), Scalar evacuates H1t/H2t.
"""

import os
import sys
import numpy as np

# concourse is normally pre-imported by the axon sitecustomize; these are
# fallbacks for environments where it is not on the default path.
if "concourse" not in sys.modules:
    try:
        import concourse  # noqa: F401
    except ImportError:
        for _p in ("/opt/trn_rl_repo", "/root/.axon_site/_ro/trn_rl_repo"):
            if os.path.isdir(_p) and _p not in sys.path:
                sys.path.append(_p)

B, N, IN_C, HID, OUT_C, N_VARS = 512, 200, 200, 256, 256, 2
N_CORES = 8
BPC = B // N_CORES  # graphs per core
P0 = 128
P1 = N - P0  # 72

# sigmoid(adj) = C_SIG * (adj + S_SIG) for adj in {0, 1}
C_SIG = float(1.0 / (1.0 + np.exp(-1.0)) - 0.5)  # 0.23105857863000487
S_SIG = float(0.5 / C_SIG)                       # 2.1639534137386535

_BUILD_CACHE = {}


def _build_nc(bpc, reps=1):
    """Build the per-core Bass program (SPMD: identical on all cores).

    reps>1 wraps the whole batch loop in a hardware For_i so the body runs
    `reps` times — benchmarking only (differencing cancels dispatch floor)."""
    import concourse.bacc as bacc
    import concourse.mybir as mybir
    import concourse.tile as tile
    from contextlib import ExitStack

    f32 = mybir.dt.float32
    f16 = mybir.dt.float16
    ADD = mybir.AluOpType.add
    MULT = mybir.AluOpType.mult

    nc = bacc.Bacc(None, target_bir_lowering=False)
    adjt = nc.declare_dram_parameter("adjt", [bpc, N, N], f16, isOutput=False)
    ft = nc.declare_dram_parameter("ft", [bpc, N, N], f16, isOutput=False)
    maska = nc.declare_dram_parameter("maska", [N, N], f16, isOutput=False)
    w0 = nc.declare_dram_parameter("w0", [IN_C, HID], f16, isOutput=False)
    w1 = nc.declare_dram_parameter("w1", [HID, HID], f16, isOutput=False)
    wp = nc.declare_dram_parameter("wp", [OUT_C, N_VARS], f16, isOutput=False)
    out = nc.declare_dram_parameter("out", [1, bpc * N_VARS], f32, isOutput=True)

    with tile.TileContext(nc) as tc, ExitStack() as ctx:
        consts = ctx.enter_context(tc.tile_pool(name="consts", bufs=1))
        inp = ctx.enter_context(tc.tile_pool(name="inp", bufs=3))
        atp = ctx.enter_context(tc.tile_pool(name="atp", bufs=2))
        sp = ctx.enter_context(tc.tile_pool(name="sp", bufs=2))
        htp = ctx.enter_context(tc.tile_pool(name="htp", bufs=2))
        smallp = ctx.enter_context(tc.tile_pool(name="smallp", bufs=2))
        pstage = ctx.enter_context(
            tc.tile_pool(name="pstage", bufs=1, space="PSUM")
        )
        psmall = ctx.enter_context(
            tc.tile_pool(name="psmall", bufs=2, space="PSUM")
        )

        # ---- constants (loaded once) ----
        w0a = consts.tile([P0, HID], f16, tag="w0a")
        w0b = consts.tile([P1, HID], f16, tag="w0b")
        w1a = consts.tile([P0, HID], f16, tag="w1a")
        w1b = consts.tile([P0, HID], f16, tag="w1b")
        wpa = consts.tile([P0, N_VARS], f16, tag="wpa")
        wpb = consts.tile([P0, N_VARS], f16, tag="wpb")
        mka = consts.tile([P0, N], f16, tag="mka")
        mkb = consts.tile([P1, N], f16, tag="mkb")
        out_acc = consts.tile([1, bpc * N_VARS], f32, tag="out_acc")
        nc.sync.dma_start(w0a[:], w0[0:P0, :])
        nc.sync.dma_start(w0b[:], w0[P0:N, :])
        nc.sync.dma_start(w1a[:], w1[0:P0, :])
        nc.sync.dma_start(w1b[:], w1[P0:HID, :])
        nc.sync.dma_start(wpa[:], wp[0:P0, :])
        nc.sync.dma_start(wpb[:], wp[P0:OUT_C, :])
        nc.sync.dma_start(mka[:], maska[0:P0, :])
        nc.sync.dma_start(mkb[:], maska[P0:N, :])
        w0_t = (w0a, w0b)
        w1_t = (w1a, w1b)
        wp_t = (wpa, wpb)

        mslc = ((0, P0), (P0, P1))  # node-dim (offset, count) tiles

        def emit_batch():
            for g in range(bpc):
                # ---- load adj^T and F^T tiles ----
                a0 = inp.tile([P0, N], f16, tag="a0")
                a1 = inp.tile([P1, N], f16, tag="a1")
                f0 = inp.tile([P0, N], f16, tag="f0")
                f1 = inp.tile([P1, N], f16, tag="f1")
                nc.sync.dma_start(a0[:], adjt[g, 0:P0, :])
                nc.sync.dma_start(a1[:], adjt[g, P0:N, :])
                nc.sync.dma_start(f0[:], ft[g, 0:P0, :])
                nc.sync.dma_start(f1[:], ft[g, P0:N, :])

                # ---- A^T = (adj^T + s) * (c*mask); w = rowsum(A^T) fused ----
                at0 = atp.tile([P0, N], f16, tag="at0")
                at1 = atp.tile([P1, N], f16, tag="at1")
                wv = atp.tile([P0, 2], f16, tag="wv")
                nc.vector.scalar_tensor_tensor(
                    at0[:], a0[:], S_SIG, mka[:], op0=ADD, op1=MULT,
                    accum_out=wv[:, 0:1],
                )
                nc.vector.scalar_tensor_tensor(
                    at1[:], a1[:], S_SIG, mkb[:], op0=ADD, op1=MULT,
                    accum_out=wv[0:P1, 1:2],
                )
                at_t = (at0, at1)

                # ---- S0 = F @ W0  -> psum [node, 2*HID] ----
                ps0 = pstage.tile([P0, 2 * HID], f32, tag="ps0")
                f_t = (f0, f1)
                for j, (mo, mc) in enumerate(mslc):
                    for k in range(2):
                        nc.tensor.matmul(
                            ps0[0:mc, j * HID:(j + 1) * HID],
                            f_t[k][:, mo:mo + mc],
                            w0_t[k][:],
                            start=(k == 0), stop=(k == 1),
                        )
                s0a = sp.tile([P0, HID], f16, tag="s0a")
                s0b = sp.tile([P1, HID], f16, tag="s0b")
                nc.vector.tensor_copy(s0a[:], ps0[:, 0:HID])
                nc.vector.tensor_copy(s0b[:], ps0[0:P1, HID:2 * HID])
                s0_t = (s0a, s0b)

                # ---- H1^T = matmul(lhsT=S0 slices, rhs=A^T) -> psum [h, 2*N] ----
                ph1 = pstage.tile([P0, 2 * N], f32, tag="ph1")
                for j in range(2):  # h slice
                    for k in range(2):  # node contraction tile
                        nc.tensor.matmul(
                            ph1[:, j * N:(j + 1) * N],
                            s0_t[k][:, j * P0:(j + 1) * P0],
                            at_t[k][:],
                            start=(k == 0), stop=(k == 1),
                        )
                h1a = htp.tile([P0, N], f16, tag="h1a")
                h1b = htp.tile([P0, N], f16, tag="h1b")
                nc.scalar.copy(h1a[:], ph1[:, 0:N])
                nc.scalar.copy(h1b[:], ph1[:, N:2 * N])
                h1_t = (h1a, h1b)

                # ---- S1 = H1 @ W1 -> psum [node, 2*HID] ----
                ps1 = pstage.tile([P0, 2 * HID], f32, tag="ps1")
                for j, (mo, mc) in enumerate(mslc):
                    for k in range(2):
                        nc.tensor.matmul(
                            ps1[0:mc, j * HID:(j + 1) * HID],
                            h1_t[k][:, mo:mo + mc],
                            w1_t[k][:],
                            start=(k == 0), stop=(k == 1),
                        )
                s1a = sp.tile([P0, HID], f16, tag="s1a")
                s1b = sp.tile([P1, HID], f16, tag="s1b")
                nc.vector.tensor_copy(s1a[:], ps1[:, 0:HID])
                nc.vector.tensor_copy(s1b[:], ps1[0:P1, HID:2 * HID])
                s1_t = (s1a, s1b)

                # ---- H2^T -> psum [h', 2*N] ----
                ph2 = pstage.tile([P0, 2 * N], f32, tag="ph2")
                for j in range(2):
                    for k in range(2):
                        nc.tensor.matmul(
                            ph2[:, j * N:(j + 1) * N],
                            s1_t[k][:, j * P0:(j + 1) * P0],
                            at_t[k][:],
                            start=(k == 0), stop=(k == 1),
                        )
                h2a = htp.tile([P0, N], f16, tag="h2a")
                h2b = htp.tile([P0, N], f16, tag="h2b")
                nc.scalar.copy(h2a[:], ph2[:, 0:N])
                nc.scalar.copy(h2b[:], ph2[:, N:2 * N])
                h2_t = (h2a, h2b)

                # ---- S2p = H2 @ Wp -> psum [node, 2*N_VARS] ----
                ps2 = psmall.tile([P0, 2 * N_VARS], f32, tag="ps2")
                for j, (mo, mc) in enumerate(mslc):
                    for k in range(2):
                        nc.tensor.matmul(
                            ps2[0:mc, j * N_VARS:(j + 1) * N_VARS],
                            h2_t[k][:, mo:mo + mc],
                            wp_t[k][:],
                            start=(k == 0), stop=(k == 1),
                        )
                s2p = smallp.tile([P0, 2 * N_VARS], f16, tag="s2p")
                nc.vector.tensor_copy(s2p[:, 0:N_VARS], ps2[:, 0:N_VARS])
                nc.vector.tensor_copy(
                    s2p[0:P1, N_VARS:2 * N_VARS], ps2[0:P1, N_VARS:2 * N_VARS]
                )

                # ---- og = w^T @ S2p -> psum [1, 2] ----
                og = psmall.tile([1, N_VARS], f32, tag="og")
                nc.tensor.matmul(
                    og[:], wv[:, 0:1], s2p[:, 0:N_VARS], start=True, stop=False
                )
                nc.tensor.matmul(
                    og[:], wv[0:P1, 1:2], s2p[0:P1, N_VARS:2 * N_VARS],
                    start=False, stop=True,
                )
                nc.vector.tensor_copy(
                    out_acc[:, g * N_VARS:(g + 1) * N_VARS], og[:]
                )

        if reps > 1:
            with tc.For_i(0, reps, 1):
                emit_batch()
        else:
            emit_batch()

        nc.sync.dma_start(out[:], out_acc[:])

    nc.compile()
    return nc


def _host_prep(adj, features, raw_edge_weight, W0, W1, W2, pw, pb):
    """Host-side weight prep + per-core input shards (fp16 on device)."""
    mask = ((raw_edge_weight + raw_edge_weight.T) * 0.5 + np.eye(N, dtype=np.float64)).astype(np.float64)
    maska = (C_SIG * mask).astype(np.float16)
    wp = (W2.astype(np.float64) @ pw.astype(np.float64) / float(N)).astype(np.float16)
    w0h = W0.astype(np.float16)
    w1h = W1.astype(np.float16)
    adjt_all = np.ascontiguousarray(
        adj.transpose(0, 2, 1).astype(np.float16)
    )
    ft_all = np.ascontiguousarray(
        features.transpose(0, 2, 1).astype(np.float16)
    )
    in_maps = []
    for c in range(N_CORES):
        sl = slice(c * BPC, (c + 1) * BPC)
        in_maps.append({
            "adjt": adjt_all[sl],
            "ft": ft_all[sl],
            "maska": maska,
            "w0": w0h,
            "w1": w1h,
            "wp": wp,
        })
    return in_maps


def _ensure_ntff_hook():
    """Wire the axon NTFF profile hook into antenv.axon_hooks if missing.

    The agent image's antenv package lacks axon_hooks, so bass_utils's
    trace path dies on import. trn_agent_boot has the ctypes hook
    implementation; expose it under the module name bass_utils expects.
    """
    import types

    try:
        from antenv.axon_hooks import get_axon_ntff_profile_hook  # noqa: F401
        return
    except ImportError:
        pass
    try:
        from trn_agent_boot.trn_boot import _ntff_profile_via_ctypes
        hook = _ntff_profile_via_ctypes("/opt/axon/libaxon_pjrt.so")
    except Exception:
        hook = None
    mod = types.ModuleType("antenv.axon_hooks")
    state = {"hook": hook}
    mod.get_axon_ntff_profile_hook = lambda: state["hook"]
    mod.set_axon_ntff_profile_hook = lambda h: state.__setitem__("hook", h)
    sys.modules["antenv.axon_hooks"] = mod
    import antenv

    antenv.axon_hooks = mod


def kernel(adj, features, raw_edge_weight, W0, W1, W2, pw, pb, _trace=False):
    from concourse.bass_utils import run_bass_kernel_spmd

    if _trace:
        _ensure_ntff_hook()

    adj = np.asarray(adj, dtype=np.float32)
    features = np.asarray(features, dtype=np.float32)
    raw_edge_weight = np.asarray(raw_edge_weight, dtype=np.float32)
    W0 = np.asarray(W0, dtype=np.float32)
    W1 = np.asarray(W1, dtype=np.float32)
    W2 = np.asarray(W2, dtype=np.float32)
    pw = np.asarray(pw, dtype=np.float32)
    pb = np.asarray(pb, dtype=np.float32)

    if "nc" not in _BUILD_CACHE:
        _BUILD_CACHE["nc"] = _build_nc(BPC)
    nc = _BUILD_CACHE["nc"]

    in_maps = _host_prep(adj, features, raw_edge_weight, W0, W1, W2, pw, pb)
    res = run_bass_kernel_spmd(
        nc, in_maps, core_ids=list(range(N_CORES)), trace=bool(_trace)
    )
    out = np.concatenate(
        [res.results[c]["out"].reshape(BPC, N_VARS) for c in range(N_CORES)], axis=0
    )
    out = out + pb[None, :].astype(np.float32)
    if _trace:
        return out, res
    return out


# revision 10
# speedup vs baseline: 2.1216x; 1.3075x over previous
"""MASKGCN Trainium2 kernel: 3-layer masked GCN over B=512 graphs of N=200 nodes.

Strategy
--------
Data-parallel over the batch: 64 graphs per NeuronCore, 8 cores, no collectives.

Math fold (exact up to fp reassociation):
    mask = (E + E^T)/2 + I                 (host)
    A    = sigmoid(adj) * mask             (device; adj is 0/1 so
                                            sigmoid(adj) = c*(adj + s), s = 0.5/c,
                                            c = sigmoid(1) - 0.5)
    S0 = F @ W0 ; H1 = A @ S0 ; S1 = H1 @ W1 ; H2 = A @ S1
    out_g = (1/200) * colsum(A)^T @ (H2 @ (W2 @ pw)) + pb
The whole last GCN layer + mean readout + linear head collapse into a
[200]-vector / [256,2]-matrix contraction (colsum(A) is a free-axis reduction
of A^T, fused into the same op that builds A^T).

Precision: all device tensors are fp16 (PE runs fp16 at 1 col/cycle vs
fp32's 4; PSUM accumulation stays fp32). Simulated end-to-end rel-norm
error vs the fp32 reference: ~9e-4.

Layouts: host passes adj^T and F^T per graph. A^T = sigmoid(adj^T) * mask
(mask symmetric). The chain alternates normal/transposed layouts so that NO
on-chip transposes are needed:
    S0  (normal  [node, h])  = matmul(lhsT=F^T slices,  rhs=W0)
    H1t (transp. [h, node])  = matmul(lhsT=S0 slices,   rhs=A^T)
    S1  (normal)             = matmul(lhsT=H1t slices,  rhs=W1)
    H2t (transp.)            = matmul(lhsT=S1 slices,   rhs=A^T)
    S2p (normal [node, 2])   = matmul(lhsT=H2t slices,  rhs=Wp)   Wp=(W2@pw)/200
    og  ([1, 2])             = matmul(lhsT=w,           rhs=S2p)  w=colsum(A)

Engine split: GpSimd builds A^T (SBUF->SBUF, fused colsum accum), Vector
evacuates S0/S1 (PSUM->SBUF), Scalar evacuates H1t/H2t.
"""

import os
import sys
import numpy as np

# concourse is normally pre-imported by the axon sitecustomize; these are
# fallbacks for environments where it is not on the default path.
if "concourse" not in sys.modules:
    try:
        import concourse  # noqa: F401
    except ImportError:
        for _p in ("/opt/trn_rl_repo", "/root/.axon_site/_ro/trn_rl_repo"):
            if os.path.isdir(_p) and _p not in sys.path:
                sys.path.append(_p)

B, N, IN_C, HID, OUT_C, N_VARS = 512, 200, 200, 256, 256, 2
N_CORES = 8
BPC = B // N_CORES  # graphs per core
P0 = 128
P1 = N - P0  # 72

# sigmoid(adj) = C_SIG * (adj + S_SIG) for adj in {0, 1}
C_SIG = float(1.0 / (1.0 + np.exp(-1.0)) - 0.5)  # 0.23105857863000487
S_SIG = float(0.5 / C_SIG)                       # 2.1639534137386535

_BUILD_CACHE = {}


def _build_nc(bpc, reps=1):
    """Build the per-core Bass program (SPMD: identical on all cores).

    reps>1 wraps the whole batch loop in a hardware For_i so the body runs
    `reps` times — benchmarking only (differencing cancels dispatch floor)."""
    import concourse.bacc as bacc
    import concourse.mybir as mybir
    import concourse.tile as tile
    from contextlib import ExitStack

    f32 = mybir.dt.float32
    f16 = mybir.dt.float16
    ADD = mybir.AluOpType.add
    MULT = mybir.AluOpType.mult

    nc = bacc.Bacc(None, target_bir_lowering=False)
    adjt = nc.declare_dram_parameter("adjt", [bpc, N, N], f16, isOutput=False)
    ft = nc.declare_dram_parameter("ft", [bpc, N, N], f16, isOutput=False)
    maska = nc.declare_dram_parameter("maska", [N, N], f16, isOutput=False)
    w0 = nc.declare_dram_parameter("w0", [IN_C, HID], f16, isOutput=False)
    w1 = nc.declare_dram_parameter("w1", [HID, HID], f16, isOutput=False)
    wp = nc.declare_dram_parameter("wp", [OUT_C, N_VARS], f16, isOutput=False)
    out = nc.declare_dram_parameter("out", [1, bpc * N_VARS], f32, isOutput=True)

    with tile.TileContext(nc) as tc, ExitStack() as ctx:
        consts = ctx.enter_context(tc.tile_pool(name="consts", bufs=1))
        inp = ctx.enter_context(tc.tile_pool(name="inp", bufs=6))
        atp = ctx.enter_context(tc.tile_pool(name="atp", bufs=3))
        sp = ctx.enter_context(tc.tile_pool(name="sp", bufs=2))
        htp = ctx.enter_context(tc.tile_pool(name="htp", bufs=2))
        smallp = ctx.enter_context(tc.tile_pool(name="smallp", bufs=2))
        pstage = ctx.enter_context(
            tc.tile_pool(name="pstage", bufs=2, space="PSUM")
        )

        # ---- constants (loaded once) ----
        w0a = consts.tile([P0, HID], f16, tag="w0a")
        w0b = consts.tile([P1, HID], f16, tag="w0b")
        w1a = consts.tile([P0, HID], f16, tag="w1a")
        w1b = consts.tile([P0, HID], f16, tag="w1b")
        wpa = consts.tile([P0, N_VARS], f16, tag="wpa")
        wpb = consts.tile([P0, N_VARS], f16, tag="wpb")
        mka = consts.tile([P0, N], f16, tag="mka")
        mkb = consts.tile([P1, N], f16, tag="mkb")
        out_acc = consts.tile([1, bpc * N_VARS], f32, tag="out_acc")
        nc.sync.dma_start(w0a[:], w0[0:P0, :])
        nc.sync.dma_start(w0b[:], w0[P0:N, :])
        nc.sync.dma_start(w1a[:], w1[0:P0, :])
        nc.sync.dma_start(w1b[:], w1[P0:HID, :])
        nc.sync.dma_start(wpa[:], wp[0:P0, :])
        nc.sync.dma_start(wpb[:], wp[P0:OUT_C, :])
        nc.sync.dma_start(mka[:], maska[0:P0, :])
        nc.sync.dma_start(mkb[:], maska[P0:N, :])
        w0_t = (w0a, w0b)
        w1_t = (w1a, w1b)
        wp_t = (wpa, wpb)

        mslc = ((0, P0), (P0, P1))  # node-dim (offset, count) tiles

        def emit_batch():
            state = {}

            def st_dma(g):
                t = {}
                t["a0"] = inp.tile([P0, N], f16, tag="a0", name="a0")
                t["a1"] = inp.tile([P1, N], f16, tag="a1", name="a1")
                t["f0"] = inp.tile([P0, N], f16, tag="f0", name="f0")
                t["f1"] = inp.tile([P1, N], f16, tag="f1", name="f1")
                nc.sync.dma_start(t["a0"][:], adjt[g, 0:P0, :])
                nc.sync.dma_start(t["a1"][:], adjt[g, P0:N, :])
                nc.sync.dma_start(t["f0"][:], ft[g, 0:P0, :])
                nc.sync.dma_start(t["f1"][:], ft[g, P0:N, :])
                state[g] = t

            def st_abuild(g):
                # A^T = (adj^T + s) * (c*mask); w = rowsum(A^T) fused
                t = state[g]
                at0 = atp.tile([P0, N], f16, tag="at0")
                at1 = atp.tile([P1, N], f16, tag="at1")
                wv = atp.tile([P0, 2], f16, tag="wv")
                nc.vector.scalar_tensor_tensor(
                    at0[:], t["a0"][:], S_SIG, mka[:], op0=ADD, op1=MULT,
                    accum_out=wv[:, 0:1],
                )
                nc.vector.scalar_tensor_tensor(
                    at1[:], t["a1"][:], S_SIG, mkb[:], op0=ADD, op1=MULT,
                    accum_out=wv[0:P1, 1:2],
                )
                t["at"] = (at0, at1)
                t["wv"] = wv

            def st_s0(g):
                # S0 = F @ W0 -> psum [node, 2*HID]; single fp16 evacuation
                t = state[g]
                ps0 = pstage.tile([P0, 2 * HID], f32, tag="ps0")
                f_t = (t["f0"], t["f1"])
                for j, (mo, mc) in enumerate(mslc):
                    for k in range(2):
                        nc.tensor.matmul(
                            ps0[0:mc, j * HID:(j + 1) * HID],
                            f_t[k][:, mo:mo + mc],
                            w0_t[k][:],
                            start=(k == 0), stop=(k == 1),
                        )
                # s01 holds [nodes0:128 x hid | nodes128:200 x hid]; the
                # copy drags along garbage rows 72:128 of the second half.
                s01 = sp.tile([P0, 2 * HID], f16, tag="s01")
                nc.vector.tensor_copy(s01[:], ps0[:])
                t["s01"] = s01

            def st_h1(g):
                # H1^T = matmul(lhsT=S0 slices, rhs=A^T) -> psum [h, 2*N]
                t = state[g]
                ph1 = pstage.tile([P0, 2 * N], f32, tag="ph1")
                s01 = t["s01"]
                for j in range(2):  # h slice
                    for k, kc in ((0, P0), (1, P1)):  # node contraction tile
                        nc.tensor.matmul(
                            ph1[:, j * N:(j + 1) * N],
                            s01[0:kc, k * HID + j * P0:k * HID + (j + 1) * P0],
                            t["at"][k][:],
                            start=(k == 0), stop=(k == 1),
                        )
                h1 = htp.tile([P0, 2 * N], f16, tag="h1")
                nc.scalar.copy(h1[:], ph1[:])
                t["h1"] = h1

            def st_s1(g):
                # S1 = H1 @ W1 -> psum [node, 2*HID]
                t = state[g]
                ps1 = pstage.tile([P0, 2 * HID], f32, tag="ps1")
                h1 = t["h1"]
                for j, (mo, mc) in enumerate(mslc):
                    for k in range(2):  # hid contraction tile
                        nc.tensor.matmul(
                            ps1[0:mc, j * HID:(j + 1) * HID],
                            h1[:, k * N + mo:k * N + mo + mc],
                            w1_t[k][:],
                            start=(k == 0), stop=(k == 1),
                        )
                s11 = sp.tile([P0, 2 * HID], f16, tag="s11")
                nc.vector.tensor_copy(s11[:], ps1[:])
                t["s11"] = s11

            def st_h2(g):
                # H2^T -> psum phx[:, 0:400]; phx also hosts S2p (400:404)
                # and og (404:406) so the whole tail shares one PSUM bank.
                t = state[g]
                phx = pstage.tile([P0, 2 * N + 2 * N_VARS + N_VARS], f32,
                                  tag="phx")
                s11 = t["s11"]
                for j in range(2):
                    for k, kc in ((0, P0), (1, P1)):
                        nc.tensor.matmul(
                            phx[:, j * N:(j + 1) * N],
                            s11[0:kc, k * HID + j * P0:k * HID + (j + 1) * P0],
                            t["at"][k][:],
                            start=(k == 0), stop=(k == 1),
                        )
                h2 = htp.tile([P0, 2 * N], f16, tag="h2")
                nc.scalar.copy(h2[:], phx[:, 0:2 * N])
                t["h2"] = h2
                t["phx"] = phx

            def st_tail(g):
                # S2p = H2 @ Wp; og = colsum(A)^T @ S2p
                t = state[g]
                phx = t["phx"]
                h2 = t["h2"]
                c0 = 2 * N
                for j, (mo, mc) in enumerate(mslc):
                    for k in range(2):  # hid contraction tile
                        nc.tensor.matmul(
                            phx[0:mc, c0 + j * N_VARS:c0 + (j + 1) * N_VARS],
                            h2[:, k * N + mo:k * N + mo + mc],
                            wp_t[k][:],
                            start=(k == 0), stop=(k == 1),
                        )
                s2p = smallp.tile([P0, 2 * N_VARS], f16, tag="s2p")
                nc.vector.tensor_copy(s2p[:], phx[:, c0:c0 + 2 * N_VARS])
                c1 = c0 + 2 * N_VARS
                wv = t["wv"]
                nc.tensor.matmul(
                    phx[0:1, c1:c1 + N_VARS], wv[:, 0:1], s2p[:, 0:N_VARS],
                    start=True, stop=False,
                )
                nc.tensor.matmul(
                    phx[0:1, c1:c1 + N_VARS], wv[0:P1, 1:2],
                    s2p[0:P1, N_VARS:2 * N_VARS],
                    start=False, stop=True,
                )
                nc.vector.tensor_copy(
                    out_acc[:, g * N_VARS:(g + 1) * N_VARS],
                    phx[0:1, c1:c1 + N_VARS],
                )
                del state[g]

            # Two-graph software pipeline: stage X of graph g overlaps
            # stage X of graph g^1, so PSUM evacuations hide under the
            # partner graph's matmul stream.
            stages = (st_s0, st_h1, st_s1, st_h2, st_tail)
            for g0 in range(0, bpc, 2):
                g1 = g0 + 1
                st_dma(g0)
                st_dma(g1)
                st_abuild(g0)
                st_abuild(g1)
                for st in stages:
                    st(g0)
                    st(g1)

        if reps > 1:
            with tc.For_i(0, reps, 1):
                emit_batch()
        else:
            emit_batch()

        nc.sync.dma_start(out[:], out_acc[:])

    nc.compile()
    return nc


def _host_prep(adj, features, raw_edge_weight, W0, W1, W2, pw, pb):
    """Host-side weight prep + per-core input shards (fp16 on device)."""
    mask = ((raw_edge_weight + raw_edge_weight.T) * 0.5 + np.eye(N, dtype=np.float64)).astype(np.float64)
    maska = (C_SIG * mask).astype(np.float16)
    wp = (W2.astype(np.float64) @ pw.astype(np.float64) / float(N)).astype(np.float16)
    w0h = W0.astype(np.float16)
    w1h = W1.astype(np.float16)
    adjt_all = np.ascontiguousarray(
        adj.transpose(0, 2, 1).astype(np.float16)
    )
    ft_all = np.ascontiguousarray(
        features.transpose(0, 2, 1).astype(np.float16)
    )
    in_maps = []
    for c in range(N_CORES):
        sl = slice(c * BPC, (c + 1) * BPC)
        in_maps.append({
            "adjt": adjt_all[sl],
            "ft": ft_all[sl],
            "maska": maska,
            "w0": w0h,
            "w1": w1h,
            "wp": wp,
        })
    return in_maps


def _ensure_ntff_hook():
    """Wire the axon NTFF profile hook into antenv.axon_hooks if missing.

    The agent image's antenv package lacks axon_hooks, so bass_utils's
    trace path dies on import. trn_agent_boot has the ctypes hook
    implementation; expose it under the module name bass_utils expects.
    """
    import types

    try:
        from antenv.axon_hooks import get_axon_ntff_profile_hook  # noqa: F401
        return
    except ImportError:
        pass
    try:
        from trn_agent_boot.trn_boot import _ntff_profile_via_ctypes
        hook = _ntff_profile_via_ctypes("/opt/axon/libaxon_pjrt.so")
    except Exception:
        hook = None
    mod = types.ModuleType("antenv.axon_hooks")
    state = {"hook": hook}
    mod.get_axon_ntff_profile_hook = lambda: state["hook"]
    mod.set_axon_ntff_profile_hook = lambda h: state.__setitem__("hook", h)
    sys.modules["antenv.axon_hooks"] = mod
    import antenv

    antenv.axon_hooks = mod


def kernel(adj, features, raw_edge_weight, W0, W1, W2, pw, pb, _trace=False):
    from concourse.bass_utils import run_bass_kernel_spmd

    if _trace:
        _ensure_ntff_hook()

    adj = np.asarray(adj, dtype=np.float32)
    features = np.asarray(features, dtype=np.float32)
    raw_edge_weight = np.asarray(raw_edge_weight, dtype=np.float32)
    W0 = np.asarray(W0, dtype=np.float32)
    W1 = np.asarray(W1, dtype=np.float32)
    W2 = np.asarray(W2, dtype=np.float32)
    pw = np.asarray(pw, dtype=np.float32)
    pb = np.asarray(pb, dtype=np.float32)

    if "nc" not in _BUILD_CACHE:
        _BUILD_CACHE["nc"] = _build_nc(BPC)
    nc = _BUILD_CACHE["nc"]

    in_maps = _host_prep(adj, features, raw_edge_weight, W0, W1, W2, pw, pb)
    res = run_bass_kernel_spmd(
        nc, in_maps, core_ids=list(range(N_CORES)), trace=bool(_trace)
    )
    out = np.concatenate(
        [res.results[c]["out"].reshape(BPC, N_VARS) for c in range(N_CORES)], axis=0
    )
    out = out + pb[None, :].astype(np.float32)
    if _trace:
        return out, res
    return out
